# revision 6
# baseline (speedup 1.0000x reference)
"""Trainium2 Bass kernel for an MoE transformer block (8 NeuronCores).

Strategy:
  Launch 1 (attention, head-parallel): core c computes q-heads {2c, 2c+1}
    (which share kv-head c//2): rmsnorm -> QKV projection (fp32r matmuls)
    -> RoPE -> causal attention -> out-projection partial (its heads'
    slice of wo). Host sums the 8 partials and adds the residual.
  Host: rmsnorm2, router softmax, top-2 routing, gates, balance loss,
    per-expert token gather (exact fp32 numpy; cheap O(T*E) work).
  Launch 2 (MoE, expert-parallel): core c holds expert c's weights and
    computes SwiGLU FFN over the tokens routed to expert c (padded to a
    common capacity). Host scatters gate-weighted outputs back.

All matmuls run in float32r (full PE throughput at free-dim >= 256,
~1e-4 relative error).
"""
import numpy as np
from contextlib import ExitStack

import concourse.bass as bass
import concourse.mybir as mybir
import concourse.tile as tile
import concourse.bacc as bacc
from concourse.bass_utils import run_bass_kernel_spmd

f32 = mybir.dt.float32
f32r = mybir.dt.float32r
AF = mybir.ActivationFunctionType
ALU = mybir.AluOpType

B, S, D = 2, 2048, 1024
T = B * S
H, KVH, HD = 16, 4, 64
E, TOPK, F = 8, 2, 4096
BAL_COEF = 0.01
EPS = 1e-6
ROPE_THETA = 10000.0
NCORES = 8

NBLK = T // 128          # 32 token blocks
SBLK = S // 128          # 16 blocks per batch
NSC = S // 512           # 4 s-chunks per batch

_TRACE = False           # set by test harness for profiling runs
_LAST_EXEC_NS = {}


# ---------------------------------------------------------------- launch 1

def build_attention(phases='ABC'):
    nc = bacc.Bacc("TRN2", target_bir_lowering=False, debug=False,
                   num_devices=NCORES)
    hid = nc.dram_tensor("hid", [T, D], f32, kind="ExternalInput")
    wqkv = nc.dram_tensor("wqkv", [D, 256], f32, kind="ExternalInput")
    wo = nc.dram_tensor("wo", [2, 64, D], f32, kind="ExternalInput")
    cos3 = nc.dram_tensor("cos3", [T, 96], f32, kind="ExternalInput")
    sin3 = nc.dram_tensor("sin3", [T, 96], f32, kind="ExternalInput")
    ident = nc.dram_tensor("ident", [128, 128], f32, kind="ExternalInput")
    mask4 = nc.dram_tensor("mask4", [128, 4, 512], f32, kind="ExternalInput")
    onesd = nc.dram_tensor("onesd", [128, 64], f32, kind="ExternalInput")
    partial = nc.dram_tensor("partial", [T, D], f32, kind="ExternalOutput")

    with tile.TileContext(nc) as tc, ExitStack() as ctx:
        const = ctx.enter_context(tc.tile_pool(name="const", bufs=1))
        big = ctx.enter_context(tc.tile_pool(name="big", bufs=1))

        id_sb = const.tile([128, 128], f32)
        nc.sync.dma_start(id_sb[:], ident[:])
        wqkv_sb = const.tile([128, 8, 256], f32r)
        nc.gpsimd.dma_start(wqkv_sb[:], wqkv.rearrange("(c p) n -> p c n", p=128))
        wo_sb = [const.tile([64, D], f32r, tag=f"wo{h}", name=f"wo_sb{h}") for h in range(2)]
        for h in range(2):
            nc.gpsimd.dma_start(wo_sb[h][:], wo[h])
        mask_sb = const.tile([128, 4, 512], f32)
        nc.sync.dma_start(mask_sb[:], mask4[:])
        ones128 = const.tile([128, 1], f32r)
        nc.gpsimd.dma_start(ones128[:], onesd[:, 0:1])
        ones1 = const.tile([1, 64], f32r)
        nc.gpsimd.dma_start(ones1[:], onesd[0:1, :])

        # persistent activations
        qT = [[big.tile([64, S], f32r, tag=f"qT{b}{h}", name=f"qT{b}{h}") for h in range(2)]
              for b in range(B)]
        kT = [big.tile([64, S], f32r, tag=f"kT{b}", name=f"kT{b}") for b in range(B)]
        v_sb = big.tile([128, NBLK, 64], f32r)
        ctxT = [[big.tile([64, S], f32r, tag=f"ctxT{b}{h}", name=f"ctxT{b}{h}") for h in range(2)]
                for b in range(B)]

        # ---------------- phase A: rmsnorm + qkv + rope + transposes
        if 'A' in phases:
         with tc.tile_pool(name="pa", bufs=3) as pool, \
             tc.tile_pool(name="pa_ps", bufs=2, space="PSUM") as psum:
            for i in range(NBLK):
                b, sblk = divmod(i, SBLK)
                x = pool.tile([128, D], f32, tag="x")
                nc.sync.dma_start(x[:], hid[i * 128:(i + 1) * 128, :])
                cs = pool.tile([128, 96], f32, tag="cs")
                nc.sync.dma_start(cs[:], cos3[i * 128:(i + 1) * 128, :])
                sn = pool.tile([128, 96], f32, tag="sn")
                nc.sync.dma_start(sn[:], sin3[i * 128:(i + 1) * 128, :])

                sq = pool.tile([128, D], f32, tag="sq")
                ssq = pool.tile([128, 1], f32, tag="ssq")
                nc.scalar.activation(sq[:], x[:], AF.Square, accum_out=ssq[:])
                var = pool.tile([128, 1], f32, tag="var")
                nc.vector.tensor_scalar(var[:], ssq[:], 1.0 / D, EPS,
                                        ALU.mult, ALU.add)
                sd = pool.tile([128, 1], f32, tag="sd")
                nc.scalar.activation(sd[:], var[:], AF.Sqrt)
                s = pool.tile([128, 1], f32, tag="s")
                nc.vector.reciprocal(s[:], sd[:])
                h_t = pool.tile([128, D], f32, tag="h")
                nc.vector.tensor_scalar_mul(h_t[:], x[:], s[:])

                hT = pool.tile([128, 8, 128], f32r, tag="hT")
                pq = psum.tile([128, 256], f32, tag="pqkv")
                for dc in range(8):
                    pt = psum.tile([128, 128], f32, tag="ptr")
                    nc.tensor.transpose(pt[:], h_t[:, dc * 128:(dc + 1) * 128],
                                        id_sb[:])
                    nc.any.tensor_copy(hT[:, dc, :], pt[:])
                for dc in range(8):
                    nc.tensor.matmul(pq[:], hT[:, dc, :], wqkv_sb[:, dc, :],
                                     start=(dc == 0), stop=(dc == 7))

                # rope on q(2 heads)+k (groups 0..2); v is group 3
                qkv_s = pool.tile([128, 256], f32, tag="qkv_s")
                nc.scalar.copy(qkv_s[:], pq[:])
                out_t = pool.tile([128, 256], f32, tag="out_t")
                g3 = qkv_s[:].rearrange("p (g n) -> p g n", g=4)[:, 0:3, :]
                o3 = out_t[:].rearrange("p (g n) -> p g n", g=4)[:, 0:3, :]
                c1, c2 = g3[:, :, 0:32], g3[:, :, 32:64]
                cs3 = cs[:].rearrange("p (g n) -> p g n", g=3)
                sn3 = sn[:].rearrange("p (g n) -> p g n", g=3)
                tmp = pool.tile([128, 3, 32], f32, tag="tmp")
                nc.vector.tensor_mul(tmp[:], c2, sn3)
                nc.vector.tensor_mul(o3[:, :, 0:32], c1, cs3)
                nc.vector.tensor_sub(o3[:, :, 0:32], o3[:, :, 0:32], tmp[:])
                tmp2 = pool.tile([128, 3, 32], f32, tag="tmp2")
                nc.vector.tensor_mul(tmp2[:], c1, sn3)
                nc.vector.tensor_mul(o3[:, :, 32:64], c2, cs3)
                nc.vector.tensor_add(o3[:, :, 32:64], o3[:, :, 32:64], tmp2[:])

                # v straight from psum
                nc.any.tensor_copy(v_sb[:, i, :], pq[:, 192:256])

                # transposes: q h0, q h1, k  ([128,64] -> [64,128])
                for h in range(2):
                    ptq = psum.tile([64, 128], f32, tag="ptq")
                    nc.tensor.transpose(ptq[:], out_t[:, h * 64:(h + 1) * 64],
                                        id_sb[:])
                    nc.any.tensor_copy(qT[b][h][:, sblk * 128:(sblk + 1) * 128],
                                       ptq[:])
                ptk = psum.tile([64, 128], f32, tag="ptk")
                nc.tensor.transpose(ptk[:], out_t[:, 128:192], id_sb[:])
                nc.any.tensor_copy(kT[b][:, sblk * 128:(sblk + 1) * 128], ptk[:])

        # ---------------- phase B: attention per (b, h, s-chunk)
        if 'B' in phases:
         with tc.tile_pool(name="pb", bufs=4) as pool, \
             tc.tile_pool(name="pb_ps", bufs=2, space="PSUM") as psum, \
             tc.tile_pool(name="pb_ps1", bufs=1, space="PSUM") as psum1:
            for b in range(B):
                for h in range(2):
                    for j in range(NSC):
                        ntb = 4 * (j + 1)
                        pctx = psum1.tile([64, 512], f32, tag="pctx")
                        prow = psum1.tile([1, 512], f32, tag="prow")
                        for k in range(ntb):
                            ps = psum.tile([128, 512], f32, tag="ps")
                            nc.tensor.matmul(
                                ps[:], kT[b][:, k * 128:(k + 1) * 128],
                                qT[b][h][:, j * 512:(j + 1) * 512],
                                start=True, stop=True)
                            ex = pool.tile([128, 512], f32r, tag="ex")
                            nc.scalar.activation(ex[:], ps[:], AF.Exp)
                            r = k - 4 * j
                            if r >= 0:  # diagonal region: apply causal mask
                                nc.vector.tensor_mul(ex[:], ex[:],
                                                     mask_sb[:, r, :])
                            nc.tensor.matmul(pctx[:], v_sb[:, b * SBLK + k, :],
                                             ex[:], start=(k == 0),
                                             stop=(k == ntb - 1))
                            nc.tensor.matmul(prow[:], ones128[:], ex[:],
                                             start=(k == 0),
                                             stop=(k == ntb - 1))
                        rec = pool.tile([1, 512], f32r, tag="rec")
                        with nc.allow_low_precision(reason="f32r rounding of softmax recip"):
                            nc.vector.reciprocal(rec[:], prow[:])
                        pbc = psum.tile([64, 512], f32, tag="pbc")
                        nc.tensor.matmul(pbc[:], ones1[:], rec[:],
                                         start=True, stop=True)
                        recb = pool.tile([64, 512], f32, tag="recb")
                        nc.scalar.copy(recb[:], pbc[:])
                        nc.vector.tensor_mul(
                            ctxT[b][h][:, j * 512:(j + 1) * 512],
                            pctx[:], recb[:])

        # ---------------- phase C: out-projection partials
        if 'C' in phases:
         with tc.tile_pool(name="pc", bufs=3) as pool, \
             tc.tile_pool(name="pc_ps", bufs=2, space="PSUM") as psum:
            for i in range(NBLK):
                b, sblk = divmod(i, SBLK)
                o_sb = pool.tile([128, D], f32, tag="o_sb")
                for dn in range(2):
                    po = psum.tile([128, 512], f32, tag="po")
                    for h in range(2):
                        nc.tensor.matmul(
                            po[:],
                            ctxT[b][h][:, sblk * 128:(sblk + 1) * 128],
                            wo_sb[h][:, dn * 512:(dn + 1) * 512],
                            start=(h == 0), stop=(h == 1))
                    nc.any.tensor_copy(o_sb[:, dn * 512:(dn + 1) * 512], po[:])
                nc.sync.dma_start(partial[i * 128:(i + 1) * 128, :], o_sb[:])

    nc.finalize()
    return nc


# ---------------------------------------------------------------- launch 2

def build_moe(cap):
    """Expert FFN: y = (silu(x@Wg) * (x@Wu)) @ Wd for `cap` tokens."""
    assert cap % 256 == 0
    ntc = cap // 256
    nc = bacc.Bacc("TRN2", target_bir_lowering=False, debug=False,
                   num_devices=NCORES)
    xT = nc.dram_tensor("xT", [D, cap], f32, kind="ExternalInput")
    wg = nc.dram_tensor("wg", [D, F], f32, kind="ExternalInput")
    wu = nc.dram_tensor("wu", [D, F], f32, kind="ExternalInput")
    wd = nc.dram_tensor("wd", [F, D], f32, kind="ExternalInput")
    y = nc.dram_tensor("y", [cap, D], f32, kind="ExternalOutput")

    NQ = 4            # F quarters
    QF = F // NQ      # 1024 f per quarter
    QFC = QF // 128   # 8 f-chunks per quarter

    with tile.TileContext(nc) as tc, ExitStack() as ctx:
        big = ctx.enter_context(tc.tile_pool(name="big", bufs=1))
        wpool = ctx.enter_context(tc.tile_pool(name="wpool", bufs=1))
        gp = ctx.enter_context(tc.tile_pool(name="gp", bufs=3))
        psg = ctx.enter_context(tc.tile_pool(name="psg", bufs=2, space="PSUM"))
        psy = ctx.enter_context(tc.tile_pool(name="psy", bufs=1, space="PSUM"))

        xT_sb = big.tile([128, 8, cap], f32r)
        nc.gpsimd.dma_start(xT_sb[:], xT.rearrange("(c p) n -> p c n", p=128))
        y_sb = big.tile([128, cap // 128, D], f32)

        for q in range(NQ):
            wg_sb = wpool.tile([128, 8, QF], f32r, tag="wg")
            nc.gpsimd.dma_start(
                wg_sb[:],
                wg[:, q * QF:(q + 1) * QF].rearrange("(c p) n -> p c n", p=128))
            wu_sb = wpool.tile([128, 8, QF], f32r, tag="wu")
            nc.gpsimd.dma_start(
                wu_sb[:],
                wu[:, q * QF:(q + 1) * QF].rearrange("(c p) n -> p c n", p=128))
            wd_sb = wpool.tile([128, QFC, D], f32r, tag="wd")
            nc.gpsimd.dma_start(
                wd_sb[:],
                wd[q * QF:(q + 1) * QF, :].rearrange("(c p) n -> p c n", p=128))

            for t in range(ntc):
                pys = [psy.tile([128, 512], f32, tag=f"py{su}{dn}", name=f"py{su}{dn}")
                       for su in range(2) for dn in range(2)]
                for fc in range(QFC):
                    pg = psg.tile([128, 256], f32, tag="pg")
                    pu = psg.tile([128, 256], f32, tag="pu")
                    for dc in range(8):
                        nc.tensor.matmul(
                            pg[:], wg_sb[:, dc, fc * 128:(fc + 1) * 128],
                            xT_sb[:, dc, t * 256:(t + 1) * 256],
                            start=(dc == 0), stop=(dc == 7))
                    for dc in range(8):
                        nc.tensor.matmul(
                            pu[:], wu_sb[:, dc, fc * 128:(fc + 1) * 128],
                            xT_sb[:, dc, t * 256:(t + 1) * 256],
                            start=(dc == 0), stop=(dc == 7))
                    sl = gp.tile([128, 256], f32, tag="sl")
                    nc.scalar.activation(sl[:], pg[:], AF.Silu)
                    gt = gp.tile([128, 256], f32r, tag="gt")
                    nc.vector.tensor_mul(gt[:], sl[:], pu[:])
                    for su in range(2):
                        for dn in range(2):
                            nc.tensor.matmul(
                                pys[su * 2 + dn][:],
                                gt[:, su * 128:(su + 1) * 128],
                                wd_sb[:, fc, dn * 512:(dn + 1) * 512],
                                start=(fc == 0), stop=(fc == QFC - 1))
                for su in range(2):
                    sub = t * 2 + su
                    for dn in range(2):
                        dst = y_sb[:, sub, dn * 512:(dn + 1) * 512]
                        if q == 0:
                            nc.any.tensor_copy(dst, pys[su * 2 + dn][:])
                        else:
                            nc.vector.tensor_add(dst, dst, pys[su * 2 + dn][:])

        for sub in range(cap // 128):
            nc.sync.dma_start(y[sub * 128:(sub + 1) * 128, :], y_sb[:, sub, :])

    nc.finalize()
    return nc


# ---------------------------------------------------------------- host glue

def _np_rmsnorm(x, w):
    var = np.mean(x * x, axis=-1, keepdims=True, dtype=np.float32)
    return (x * (1.0 / np.sqrt(var + EPS)) * w).astype(np.float32)


def _softmax(x):
    m = x.max(axis=-1, keepdims=True)
    e = np.exp((x - m).astype(np.float32))
    return e / e.sum(axis=-1, keepdims=True, dtype=np.float32)


_ATTN_NC = None
_MOE_NC = {}


def _get_attn_nc():
    global _ATTN_NC
    if _ATTN_NC is None:
        _ATTN_NC = build_attention()
    return _ATTN_NC


def _get_moe_nc(cap):
    if cap not in _MOE_NC:
        _MOE_NC[cap] = build_moe(cap)
    return _MOE_NC[cap]


def kernel(hidden_states, ln1_w, wq, wk, wv, wo, ln2_w, router_w,
           w_gate, w_up, w_down):
    global _LAST_EXEC_NS
    hidden_states = np.asarray(hidden_states, dtype=np.float32)
    ln1_w = np.asarray(ln1_w, dtype=np.float32)
    wq = np.asarray(wq, dtype=np.float32)
    wk = np.asarray(wk, dtype=np.float32)
    wv = np.asarray(wv, dtype=np.float32)
    wo = np.asarray(wo, dtype=np.float32)
    ln2_w = np.asarray(ln2_w, dtype=np.float32)
    router_w = np.asarray(router_w, dtype=np.float32)
    w_gate = np.asarray(w_gate, dtype=np.float32)
    w_up = np.asarray(w_up, dtype=np.float32)
    w_down = np.asarray(w_down, dtype=np.float32)

    hid = hidden_states.reshape(T, D)

    # ------- launch 1: attention
    pos = np.arange(S, dtype=np.float32)
    inv = 1.0 / (ROPE_THETA ** (np.arange(0, HD, 2, dtype=np.float32) / HD))
    ang = pos[:, None] * inv[None, :]
    cosT = np.tile(np.cos(ang).astype(np.float32), (B, 1))
    sinT = np.tile(np.sin(ang).astype(np.float32), (B, 1))
    qs = HD ** -0.5  # fold score scale into q rope tables
    cos3 = np.concatenate([cosT * qs, cosT * qs, cosT], 1).astype(np.float32)
    sin3 = np.concatenate([sinT * qs, sinT * qs, sinT], 1).astype(np.float32)
    ident = np.eye(128, dtype=np.float32)
    # mask4[i, r, j] = 1 if (r*128 + i) <= j  (t-block r within s-chunk)
    ii = np.arange(128)[:, None, None]
    rr = np.arange(4)[None, :, None]
    jj = np.arange(512)[None, None, :]
    mask4 = ((rr * 128 + ii) <= jj).astype(np.float32)

    wql = ln1_w[:, None] * wq
    wkl = ln1_w[:, None] * wk
    wvl = ln1_w[:, None] * wv

    in_maps = []
    for c in range(NCORES):
        kvh = c // 2
        wqkv_c = np.ascontiguousarray(np.concatenate([
            wql[:, c * 128:(c + 1) * 128],
            wkl[:, kvh * 64:(kvh + 1) * 64],
            wvl[:, kvh * 64:(kvh + 1) * 64]], axis=1))
        wo_c = np.ascontiguousarray(
            wo[c * 128:(c + 1) * 128, :].reshape(2, 64, D))
        in_maps.append(dict(hid=hid, wqkv=wqkv_c, wo=wo_c, cos3=cos3,
                            sin3=sin3, ident=ident, mask4=mask4,
                            onesd=np.ones((128, 64), np.float32)))

    nc1 = _get_attn_nc()
    res1 = run_bass_kernel_spmd(nc1, in_maps, core_ids=list(range(NCORES)),
                                trace=_TRACE)
    _LAST_EXEC_NS["attn"] = res1.exec_time_ns
    attn_out = res1.results[0]["partial"].copy()
    for c in range(1, NCORES):
        attn_out += res1.results[c]["partial"]

    hid2 = hid + attn_out

    # ------- host routing (cheap, exact fp32)
    h2 = _np_rmsnorm(hid2, ln2_w)
    logits = h2 @ router_w.T                      # [T, E]
    probs = _softmax(logits)
    top_i = np.argsort(-probs, axis=-1, kind="stable")[:, :TOPK]
    top_p = np.take_along_axis(probs, top_i, axis=-1)
    top_p = top_p / top_p.sum(axis=-1, keepdims=True)

    sel = np.zeros((T, E), dtype=np.float32)
    np.add.at(sel, (np.arange(T)[:, None], top_i), 1.0)
    frac = sel.mean(axis=0) / TOPK
    prob_mean = probs.mean(axis=0)
    balance_loss = np.float32(BAL_COEF * E * np.sum(frac * prob_mean))

    # ------- launch 2: expert FFN
    idx = [np.where(sel[:, e] > 0)[0] for e in range(E)]
    maxc = max(len(ix) for ix in idx)
    cap = max(256, ((maxc + 255) // 256) * 256)

    in_maps2 = []
    for e in range(E):
        xg = np.zeros((cap, D), dtype=np.float32)
        xg[:len(idx[e])] = h2[idx[e]]
        in_maps2.append(dict(
            xT=np.ascontiguousarray(xg.T),
            wg=np.ascontiguousarray(w_gate[e]),
            wu=np.ascontiguousarray(w_up[e]),
            wd=np.ascontiguousarray(w_down[e])))

    nc2 = _get_moe_nc(cap)
    res2 = run_bass_kernel_spmd(nc2, in_maps2, core_ids=list(range(NCORES)),
                                trace=_TRACE)
    _LAST_EXEC_NS["moe"] = res2.exec_time_ns

    gates = np.zeros((T, E), dtype=np.float32)
    np.put_along_axis(gates, top_i, top_p.astype(np.float32), axis=-1)
    moe_out = np.zeros((T, D), dtype=np.float32)
    for e in range(E):
        ye = res2.results[e]["y"][:len(idx[e])]
        moe_out[idx[e]] += gates[idx[e], e][:, None] * ye

    out = (hid2 + moe_out).reshape(B, S, D).astype(np.float32)
    return out, balance_loss


# revision 10
# speedup vs baseline: 1.0792x; 1.0792x over previous
"""Trainium2 Bass kernel for an MoE transformer block (8 NeuronCores).

Strategy:
  Launch 1 (attention, head-parallel): core c computes q-heads {2c, 2c+1}
    (which share kv-head c//2): rmsnorm -> QKV projection (fp32r matmuls)
    -> RoPE -> causal attention -> out-projection partial (its heads'
    slice of wo). Host sums the 8 partials and adds the residual.
  Host: rmsnorm2, router softmax, top-2 routing, gates, balance loss,
    per-expert token gather (exact fp32 numpy; cheap O(T*E) work).
  Launch 2 (MoE, expert-parallel): core c holds expert c's weights and
    computes SwiGLU FFN over the tokens routed to expert c (padded to a
    common capacity). Host scatters gate-weighted outputs back.

All matmuls run in float32r (full PE throughput at free-dim >= 256,
~1e-4 relative error).
"""
import numpy as np
from contextlib import ExitStack

import concourse.bass as bass
import concourse.mybir as mybir
import concourse.tile as tile
import concourse.bacc as bacc
from concourse.bass_utils import run_bass_kernel_spmd

f32 = mybir.dt.float32
f32r = mybir.dt.float32r
AF = mybir.ActivationFunctionType
ALU = mybir.AluOpType

B, S, D = 2, 2048, 1024
T = B * S
H, KVH, HD = 16, 4, 64
E, TOPK, F = 8, 2, 4096
BAL_COEF = 0.01
EPS = 1e-6
ROPE_THETA = 10000.0
NCORES = 8

NBLK = T // 128          # 32 token blocks
SBLK = S // 128          # 16 blocks per batch
NSC = S // 512           # 4 s-chunks per batch

_TRACE = False           # set by test harness for profiling runs
_LAST_EXEC_NS = {}


# ---------------------------------------------------------------- launch 1

def build_attention(phases='ABC'):
    nc = bacc.Bacc("TRN2", target_bir_lowering=False, debug=False,
                   num_devices=NCORES)
    hid = nc.dram_tensor("hid", [T, D], f32, kind="ExternalInput")
    wqkv = nc.dram_tensor("wqkv", [D, 256], f32, kind="ExternalInput")
    wo = nc.dram_tensor("wo", [2, 64, D], f32, kind="ExternalInput")
    cos3 = nc.dram_tensor("cos3", [T, 96], f32, kind="ExternalInput")
    sin3 = nc.dram_tensor("sin3", [T, 96], f32, kind="ExternalInput")
    ident = nc.dram_tensor("ident", [128, 128], f32, kind="ExternalInput")
    mask4 = nc.dram_tensor("mask4", [128, 4, 512], f32, kind="ExternalInput")
    onesd = nc.dram_tensor("onesd", [128, 64], f32, kind="ExternalInput")
    partial = nc.dram_tensor("partial", [T, D], f32, kind="ExternalOutput")

    with tile.TileContext(nc) as tc, ExitStack() as ctx:
        const = ctx.enter_context(tc.tile_pool(name="const", bufs=1))
        big = ctx.enter_context(tc.tile_pool(name="big", bufs=1))

        id_sb = const.tile([128, 128], f32r)
        nc.gpsimd.dma_start(id_sb[:], ident[:])
        wqkv_sb = const.tile([128, 8, 256], f32r)
        nc.gpsimd.dma_start(wqkv_sb[:], wqkv.rearrange("(c p) n -> p c n", p=128))
        wo_sb = [const.tile([64, D], f32r, tag=f"wo{h}", name=f"wo_sb{h}") for h in range(2)]
        for h in range(2):
            nc.gpsimd.dma_start(wo_sb[h][:], wo[h])
        mask_sb = const.tile([128, 4, 512], f32)
        nc.sync.dma_start(mask_sb[:], mask4[:])
        ones_sb = const.tile([65, 64], f32r)
        nc.gpsimd.dma_start(ones_sb[:], onesd[0:65, :])

        # persistent activations
        qT = [[big.tile([64, S], f32r, tag=f"qT{b}{h}", name=f"qT{b}{h}") for h in range(2)]
              for b in range(B)]
        kT = [big.tile([64, S], f32r, tag=f"kT{b}", name=f"kT{b}") for b in range(B)]
        v_sb = big.tile([128, NBLK, 65], f32r)
        nc.gpsimd.dma_start(v_sb[:, :, 64:65], onesd[:, 0:NBLK].unsqueeze(2))
        ctxT = [[big.tile([64, S], f32r, tag=f"ctxT{b}{h}", name=f"ctxT{b}{h}") for h in range(2)]
                for b in range(B)]

        # ---------------- phase A: rmsnorm + qkv + rope + transposes
        if 'A' in phases:
         with tc.tile_pool(name="pa", bufs=3) as pool, \
             tc.tile_pool(name="pa_ps", bufs=2, space="PSUM") as psum:
            for i in range(NBLK):
                b, sblk = divmod(i, SBLK)
                x = pool.tile([128, D], f32, tag="x")
                nc.sync.dma_start(x[:], hid[i * 128:(i + 1) * 128, :])
                cs = pool.tile([128, 96], f32, tag="cs")
                nc.sync.dma_start(cs[:], cos3[i * 128:(i + 1) * 128, :])
                sn = pool.tile([128, 96], f32, tag="sn")
                nc.sync.dma_start(sn[:], sin3[i * 128:(i + 1) * 128, :])

                sq = pool.tile([128, D], f32, tag="sq")
                ssq = pool.tile([128, 1], f32, tag="ssq")
                nc.scalar.activation(sq[:], x[:], AF.Square, accum_out=ssq[:])
                var = pool.tile([128, 1], f32, tag="var")
                nc.vector.tensor_scalar(var[:], ssq[:], 1.0 / D, EPS,
                                        ALU.mult, ALU.add)
                sd = pool.tile([128, 1], f32, tag="sd")
                nc.scalar.activation(sd[:], var[:], AF.Sqrt)
                s = pool.tile([128, 1], f32, tag="s")
                nc.vector.reciprocal(s[:], sd[:])
                h_t = pool.tile([128, D], f32r, tag="h")
                nc.vector.tensor_scalar_mul(h_t[:], x[:], s[:])

                hT = pool.tile([128, 8, 128], f32r, tag="hT")
                pq = psum.tile([128, 256], f32, tag="pqkv")
                for half in range(2):
                    pt = psum.tile([128, 512], f32r, tag="ptr")
                    for q4 in range(4):
                        dc = half * 4 + q4
                        nc.tensor.transpose(pt[:, q4 * 128:(q4 + 1) * 128],
                                            h_t[:, dc * 128:(dc + 1) * 128],
                                            id_sb[:])
                    nc.vector.tensor_copy(
                        hT[:, half * 4:(half + 1) * 4, :],
                        pt[:].rearrange("p (c n) -> p c n", c=4))
                for dc in range(8):
                    nc.tensor.matmul(pq[:], hT[:, dc, :], wqkv_sb[:, dc, :],
                                     start=(dc == 0), stop=(dc == 7))

                # rope on q(2 heads)+k (groups 0..2); v is group 3
                out_t = pool.tile([128, 256], f32r, tag="out_t")
                g3 = pq[:].rearrange("p (g n) -> p g n", g=4)[:, 0:3, :]
                o3 = out_t[:].rearrange("p (g n) -> p g n", g=4)[:, 0:3, :]
                c1, c2 = g3[:, :, 0:32], g3[:, :, 32:64]
                cs3 = cs[:].rearrange("p (g n) -> p g n", g=3)
                sn3 = sn[:].rearrange("p (g n) -> p g n", g=3)
                tmp = pool.tile([128, 3, 32], f32, tag="tmp")
                nc.vector.tensor_mul(tmp[:], c2, sn3)
                nc.vector.tensor_mul(o3[:, :, 0:32], c1, cs3)
                nc.vector.tensor_sub(o3[:, :, 0:32], o3[:, :, 0:32], tmp[:])
                tmp2 = pool.tile([128, 3, 32], f32, tag="tmp2")
                nc.vector.tensor_mul(tmp2[:], c1, sn3)
                nc.vector.tensor_mul(o3[:, :, 32:64], c2, cs3)
                nc.vector.tensor_add(o3[:, :, 32:64], o3[:, :, 32:64], tmp2[:])

                # v straight from psum
                nc.any.tensor_copy(v_sb[:, i, 0:64], pq[:, 192:256])

                # transposes: q h0, q h1, k  ([128,64] -> [64,128])
                for h in range(2):
                    ptq = psum.tile([64, 128], f32r, tag="ptq")
                    nc.tensor.transpose(ptq[:], out_t[:, h * 64:(h + 1) * 64],
                                        id_sb[:])
                    nc.any.tensor_copy(qT[b][h][:, sblk * 128:(sblk + 1) * 128],
                                       ptq[:])
                ptk = psum.tile([64, 128], f32r, tag="ptk")
                nc.tensor.transpose(ptk[:], out_t[:, 128:192], id_sb[:])
                nc.any.tensor_copy(kT[b][:, sblk * 128:(sblk + 1) * 128], ptk[:])

        # ---------------- phase B: attention per (b, h, s-chunk)
        if 'B' in phases:
         with tc.tile_pool(name="pb", bufs=4) as pool, \
             tc.tile_pool(name="pb_ps", bufs=2, space="PSUM") as psum, \
             tc.tile_pool(name="pb_ps1", bufs=1, space="PSUM") as psum1:
            for b in range(B):
                for h in range(2):
                    for j in range(NSC):
                        ntb = 4 * (j + 1)
                        pctx = psum1.tile([65, 512], f32, tag="pctx")
                        for k in range(ntb):
                            ps = psum.tile([128, 512], f32, tag="ps")
                            nc.tensor.matmul(
                                ps[:], kT[b][:, k * 128:(k + 1) * 128],
                                qT[b][h][:, j * 512:(j + 1) * 512],
                                start=True, stop=True)
                            ex = pool.tile([128, 512], f32r, tag="ex")
                            nc.scalar.activation(ex[:], ps[:], AF.Exp)
                            r = k - 4 * j
                            if r >= 0:  # diagonal region: apply causal mask
                                nc.vector.tensor_mul(ex[:], ex[:],
                                                     mask_sb[:, r, :])
                            nc.tensor.matmul(pctx[:], v_sb[:, b * SBLK + k, :],
                                             ex[:], start=(k == 0),
                                             stop=(k == ntb - 1))
                        rec = pool.tile([65, 512], f32r, tag="rec")
                        with nc.allow_low_precision(reason="f32r rounding of softmax recip"):
                            nc.vector.reciprocal(rec[64:65, :], pctx[64:65, :])
                        pbc = psum.tile([64, 512], f32, tag="pbc")
                        nc.tensor.matmul(pbc[:], ones_sb[64:65, :], rec[64:65, :],
                                         start=True, stop=True)
                        recb = pool.tile([64, 512], f32, tag="recb")
                        nc.scalar.copy(recb[:], pbc[:])
                        nc.vector.tensor_mul(
                            ctxT[b][h][:, j * 512:(j + 1) * 512],
                            pctx[0:64, :], recb[:])

        # ---------------- phase C: out-projection partials
        if 'C' in phases:
         with tc.tile_pool(name="pc", bufs=3) as pool, \
             tc.tile_pool(name="pc_ps", bufs=2, space="PSUM") as psum:
            for i in range(NBLK):
                b, sblk = divmod(i, SBLK)
                o_sb = pool.tile([128, D], f32, tag="o_sb")
                for dn in range(2):
                    po = psum.tile([128, 512], f32, tag="po")
                    for h in range(2):
                        nc.tensor.matmul(
                            po[:],
                            ctxT[b][h][:, sblk * 128:(sblk + 1) * 128],
                            wo_sb[h][:, dn * 512:(dn + 1) * 512],
                            start=(h == 0), stop=(h == 1))
                    nc.any.tensor_copy(o_sb[:, dn * 512:(dn + 1) * 512], po[:])
                nc.sync.dma_start(partial[i * 128:(i + 1) * 128, :], o_sb[:])

    nc.finalize()
    return nc


# ---------------------------------------------------------------- launch 2

def build_moe(cap):
    """Expert FFN: y = (silu(x@Wg) * (x@Wu)) @ Wd for `cap` tokens."""
    assert cap % 256 == 0
    ntc = cap // 256
    nc = bacc.Bacc("TRN2", target_bir_lowering=False, debug=False,
                   num_devices=NCORES)
    xT = nc.dram_tensor("xT", [D, cap], f32, kind="ExternalInput")
    wg = nc.dram_tensor("wg", [D, F], f32, kind="ExternalInput")
    wu = nc.dram_tensor("wu", [D, F], f32, kind="ExternalInput")
    wd = nc.dram_tensor("wd", [F, D], f32, kind="ExternalInput")
    y = nc.dram_tensor("y", [cap, D], f32, kind="ExternalOutput")

    NQ = 8            # F octants
    QF = F // NQ      # 512 f per octant
    QFC = QF // 128   # 4 f-chunks per octant

    with tile.TileContext(nc) as tc, ExitStack() as ctx:
        big = ctx.enter_context(tc.tile_pool(name="big", bufs=1))
        wpool = ctx.enter_context(tc.tile_pool(name="wpool", bufs=2))
        gp = ctx.enter_context(tc.tile_pool(name="gp", bufs=3))
        psg = ctx.enter_context(tc.tile_pool(name="psg", bufs=2, space="PSUM"))
        psy = ctx.enter_context(tc.tile_pool(name="psy", bufs=1, space="PSUM"))

        xT_sb = big.tile([128, 8, cap], f32r)
        nc.gpsimd.dma_start(xT_sb[:], xT.rearrange("(c p) n -> p c n", p=128))
        y_sb = big.tile([128, cap // 128, D], f32)

        for q in range(NQ):
            wg_sb = wpool.tile([128, 8, QF], f32r, tag="wg")
            nc.gpsimd.dma_start(
                wg_sb[:],
                wg[:, q * QF:(q + 1) * QF].rearrange("(c p) n -> p c n", p=128))
            wu_sb = wpool.tile([128, 8, QF], f32r, tag="wu")
            nc.gpsimd.dma_start(
                wu_sb[:],
                wu[:, q * QF:(q + 1) * QF].rearrange("(c p) n -> p c n", p=128))
            wd_sb = wpool.tile([128, QFC, D], f32r, tag="wd")
            nc.gpsimd.dma_start(
                wd_sb[:],
                wd[q * QF:(q + 1) * QF, :].rearrange("(c p) n -> p c n", p=128))

            for t in range(ntc):
                pys = [psy.tile([128, 512], f32, tag=f"py{su}{dn}", name=f"py{su}{dn}")
                       for su in range(2) for dn in range(2)]
                for fc in range(QFC):
                    pg = psg.tile([128, 256], f32, tag="pg")
                    pu = psg.tile([128, 256], f32, tag="pu")
                    for dc in range(8):
                        nc.tensor.matmul(
                            pg[:], wg_sb[:, dc, fc * 128:(fc + 1) * 128],
                            xT_sb[:, dc, t * 256:(t + 1) * 256],
                            start=(dc == 0), stop=(dc == 7))
                    for dc in range(8):
                        nc.tensor.matmul(
                            pu[:], wu_sb[:, dc, fc * 128:(fc + 1) * 128],
                            xT_sb[:, dc, t * 256:(t + 1) * 256],
                            start=(dc == 0), stop=(dc == 7))
                    sl = gp.tile([128, 256], f32, tag="sl")
                    nc.scalar.activation(sl[:], pg[:], AF.Silu)
                    gt = gp.tile([128, 256], f32r, tag="gt")
                    nc.vector.tensor_mul(gt[:], sl[:], pu[:])
                    for su in range(2):
                        for dn in range(2):
                            nc.tensor.matmul(
                                pys[su * 2 + dn][:],
                                gt[:, su * 128:(su + 1) * 128],
                                wd_sb[:, fc, dn * 512:(dn + 1) * 512],
                                start=(fc == 0), stop=(fc == QFC - 1))
                for su in range(2):
                    sub = t * 2 + su
                    for dn in range(2):
                        dst = y_sb[:, sub, dn * 512:(dn + 1) * 512]
                        if q == 0:
                            nc.any.tensor_copy(dst, pys[su * 2 + dn][:])
                        else:
                            nc.vector.tensor_add(dst, dst, pys[su * 2 + dn][:])

        for sub in range(cap // 128):
            nc.sync.dma_start(y[sub * 128:(sub + 1) * 128, :], y_sb[:, sub, :])

    nc.finalize()
    return nc


# ---------------------------------------------------------------- host glue

def _np_rmsnorm(x, w):
    var = np.mean(x * x, axis=-1, keepdims=True, dtype=np.float32)
    return (x * (1.0 / np.sqrt(var + EPS)) * w).astype(np.float32)


def _softmax(x):
    m = x.max(axis=-1, keepdims=True)
    e = np.exp((x - m).astype(np.float32))
    return e / e.sum(axis=-1, keepdims=True, dtype=np.float32)


_ATTN_NC = None
_MOE_NC = {}


def _get_attn_nc():
    global _ATTN_NC
    if _ATTN_NC is None:
        _ATTN_NC = build_attention()
    return _ATTN_NC


def _get_moe_nc(cap):
    if cap not in _MOE_NC:
        _MOE_NC[cap] = build_moe(cap)
    return _MOE_NC[cap]


def kernel(hidden_states, ln1_w, wq, wk, wv, wo, ln2_w, router_w,
           w_gate, w_up, w_down):
    global _LAST_EXEC_NS
    hidden_states = np.asarray(hidden_states, dtype=np.float32)
    ln1_w = np.asarray(ln1_w, dtype=np.float32)
    wq = np.asarray(wq, dtype=np.float32)
    wk = np.asarray(wk, dtype=np.float32)
    wv = np.asarray(wv, dtype=np.float32)
    wo = np.asarray(wo, dtype=np.float32)
    ln2_w = np.asarray(ln2_w, dtype=np.float32)
    router_w = np.asarray(router_w, dtype=np.float32)
    w_gate = np.asarray(w_gate, dtype=np.float32)
    w_up = np.asarray(w_up, dtype=np.float32)
    w_down = np.asarray(w_down, dtype=np.float32)

    hid = hidden_states.reshape(T, D)

    # ------- launch 1: attention
    pos = np.arange(S, dtype=np.float32)
    inv = 1.0 / (ROPE_THETA ** (np.arange(0, HD, 2, dtype=np.float32) / HD))
    ang = pos[:, None] * inv[None, :]
    cosT = np.tile(np.cos(ang).astype(np.float32), (B, 1))
    sinT = np.tile(np.sin(ang).astype(np.float32), (B, 1))
    qs = HD ** -0.5  # fold score scale into q rope tables
    cos3 = np.concatenate([cosT * qs, cosT * qs, cosT], 1).astype(np.float32)
    sin3 = np.concatenate([sinT * qs, sinT * qs, sinT], 1).astype(np.float32)
    ident = np.eye(128, dtype=np.float32)
    # mask4[i, r, j] = 1 if (r*128 + i) <= j  (t-block r within s-chunk)
    ii = np.arange(128)[:, None, None]
    rr = np.arange(4)[None, :, None]
    jj = np.arange(512)[None, None, :]
    mask4 = ((rr * 128 + ii) <= jj).astype(np.float32)

    wql = ln1_w[:, None] * wq
    wkl = ln1_w[:, None] * wk
    wvl = ln1_w[:, None] * wv

    in_maps = []
    for c in range(NCORES):
        kvh = c // 2
        wqkv_c = np.ascontiguousarray(np.concatenate([
            wql[:, c * 128:(c + 1) * 128],
            wkl[:, kvh * 64:(kvh + 1) * 64],
            wvl[:, kvh * 64:(kvh + 1) * 64]], axis=1))
        wo_c = np.ascontiguousarray(
            wo[c * 128:(c + 1) * 128, :].reshape(2, 64, D))
        in_maps.append(dict(hid=hid, wqkv=wqkv_c, wo=wo_c, cos3=cos3,
                            sin3=sin3, ident=ident, mask4=mask4,
                            onesd=np.ones((128, 64), np.float32)))

    nc1 = _get_attn_nc()
    res1 = run_bass_kernel_spmd(nc1, in_maps, core_ids=list(range(NCORES)),
                                trace=_TRACE)
    _LAST_EXEC_NS["attn"] = res1.exec_time_ns
    attn_out = res1.results[0]["partial"].copy()
    for c in range(1, NCORES):
        attn_out += res1.results[c]["partial"]

    hid2 = hid + attn_out

    # ------- host routing (cheap, exact fp32)
    h2 = _np_rmsnorm(hid2, ln2_w)
    logits = h2 @ router_w.T                      # [T, E]
    probs = _softmax(logits)
    top_i = np.argsort(-probs, axis=-1, kind="stable")[:, :TOPK]
    top_p = np.take_along_axis(probs, top_i, axis=-1)
    top_p = top_p / top_p.sum(axis=-1, keepdims=True)

    sel = np.zeros((T, E), dtype=np.float32)
    np.add.at(sel, (np.arange(T)[:, None], top_i), 1.0)
    frac = sel.mean(axis=0) / TOPK
    prob_mean = probs.mean(axis=0)
    balance_loss = np.float32(BAL_COEF * E * np.sum(frac * prob_mean))

    # ------- launch 2: expert FFN
    idx = [np.where(sel[:, e] > 0)[0] for e in range(E)]
    maxc = max(len(ix) for ix in idx)
    cap = max(256, ((maxc + 255) // 256) * 256)

    in_maps2 = []
    for e in range(E):
        xg = np.zeros((cap, D), dtype=np.float32)
        xg[:len(idx[e])] = h2[idx[e]]
        in_maps2.append(dict(
            xT=np.ascontiguousarray(xg.T),
            wg=np.ascontiguousarray(w_gate[e]),
            wu=np.ascontiguousarray(w_up[e]),
            wd=np.ascontiguousarray(w_down[e])))

    nc2 = _get_moe_nc(cap)
    res2 = run_bass_kernel_spmd(nc2, in_maps2, core_ids=list(range(NCORES)),
                                trace=_TRACE)
    _LAST_EXEC_NS["moe"] = res2.exec_time_ns

    gates = np.zeros((T, E), dtype=np.float32)
    np.put_along_axis(gates, top_i, top_p.astype(np.float32), axis=-1)
    moe_out = np.zeros((T, D), dtype=np.float32)
    for e in range(E):
        ye = res2.results[e]["y"][:len(idx[e])]
        moe_out[idx[e]] += gates[idx[e], e][:, None] * ye

    out = (hid2 + moe_out).reshape(B, S, D).astype(np.float32)
    return out, balance_loss


# revision 11
# speedup vs baseline: 1.1010x; 1.0202x over previous
"""Trainium2 Bass kernel for an MoE transformer block (8 NeuronCores).

Strategy:
  Launch 1 (attention, head-parallel): core c computes q-heads {2c, 2c+1}
    (which share kv-head c//2): rmsnorm -> QKV projection (fp32r matmuls)
    -> RoPE -> causal attention -> out-projection partial (its heads'
    slice of wo). Host sums the 8 partials and adds the residual.
  Host: rmsnorm2, router softmax, top-2 routing, gates, balance loss,
    per-expert token gather (exact fp32 numpy; cheap O(T*E) work).
  Launch 2 (MoE, expert-parallel): core c holds expert c's weights and
    computes SwiGLU FFN over the tokens routed to expert c (padded to a
    common capacity). Host scatters gate-weighted outputs back.

All matmuls run in float32r (full PE throughput at free-dim >= 256,
~1e-4 relative error).
"""
import numpy as np
from contextlib import ExitStack

import concourse.bass as bass
import concourse.mybir as mybir
import concourse.tile as tile
import concourse.bacc as bacc
from concourse.bass_utils import run_bass_kernel_spmd

f32 = mybir.dt.float32
f32r = mybir.dt.float32r
AF = mybir.ActivationFunctionType
ALU = mybir.AluOpType

B, S, D = 2, 2048, 1024
T = B * S
H, KVH, HD = 16, 4, 64
E, TOPK, F = 8, 2, 4096
BAL_COEF = 0.01
EPS = 1e-6
ROPE_THETA = 10000.0
NCORES = 8

NBLK = T // 128          # 32 token blocks
SBLK = S // 128          # 16 blocks per batch
NSC = S // 512           # 4 s-chunks per batch

_TRACE = False           # set by test harness for profiling runs
_LAST_EXEC_NS = {}


# ---------------------------------------------------------------- launch 1

def build_attention(phases='ABC'):
    nc = bacc.Bacc("TRN2", target_bir_lowering=False, debug=False,
                   num_devices=NCORES)
    hid = nc.dram_tensor("hid", [T, D], f32, kind="ExternalInput")
    wqkv = nc.dram_tensor("wqkv", [D, 256], f32, kind="ExternalInput")
    wo = nc.dram_tensor("wo", [2, 64, D], f32, kind="ExternalInput")
    cos3 = nc.dram_tensor("cos3", [T, 96], f32, kind="ExternalInput")
    sin3 = nc.dram_tensor("sin3", [T, 96], f32, kind="ExternalInput")
    ident = nc.dram_tensor("ident", [128, 128], f32, kind="ExternalInput")
    mask4 = nc.dram_tensor("mask4", [128, 4, 512], f32, kind="ExternalInput")
    onesd = nc.dram_tensor("onesd", [128, 64], f32, kind="ExternalInput")
    partial = nc.dram_tensor("partial", [T, D], f32, kind="ExternalOutput")

    with tile.TileContext(nc) as tc, ExitStack() as ctx:
        const = ctx.enter_context(tc.tile_pool(name="const", bufs=1))
        big = ctx.enter_context(tc.tile_pool(name="big", bufs=1))

        id_sb = const.tile([128, 128], f32r)
        nc.gpsimd.dma_start(id_sb[:], ident[:])
        wqkv_sb = const.tile([128, 8, 256], f32r)
        nc.gpsimd.dma_start(wqkv_sb[:], wqkv.rearrange("(c p) n -> p c n", p=128))
        wo_sb = [const.tile([64, D], f32r, tag=f"wo{h}", name=f"wo_sb{h}") for h in range(2)]
        for h in range(2):
            nc.gpsimd.dma_start(wo_sb[h][:], wo[h])
        mask_sb = const.tile([128, 4, 512], f32)
        nc.sync.dma_start(mask_sb[:], mask4[:])
        ones_sb = const.tile([65, 64], f32r)
        nc.gpsimd.dma_start(ones_sb[:], onesd[0:65, :])

        # persistent activations
        qT = [[big.tile([64, S], f32r, tag=f"qT{b}{h}", name=f"qT{b}{h}") for h in range(2)]
              for b in range(B)]
        kT = [big.tile([64, S], f32r, tag=f"kT{b}", name=f"kT{b}") for b in range(B)]
        v_sb = big.tile([128, NBLK, 65], f32r)
        nc.gpsimd.dma_start(v_sb[:, :, 64:65], onesd[:, 0:NBLK].unsqueeze(2))
        ctxT = [[big.tile([64, S], f32r, tag=f"ctxT{b}{h}", name=f"ctxT{b}{h}") for h in range(2)]
                for b in range(B)]

        # ---------------- phase A: rmsnorm + qkv + rope + transposes
        if 'A' in phases:
         with tc.tile_pool(name="pa", bufs=3) as pool, \
             tc.tile_pool(name="pa_ps", bufs=2, space="PSUM") as psum:
            for i in range(NBLK):
                b, sblk = divmod(i, SBLK)
                x = pool.tile([128, D], f32, tag="x")
                nc.sync.dma_start(x[:], hid[i * 128:(i + 1) * 128, :])
                cs = pool.tile([128, 96], f32, tag="cs")
                nc.sync.dma_start(cs[:], cos3[i * 128:(i + 1) * 128, :])
                sn = pool.tile([128, 96], f32, tag="sn")
                nc.sync.dma_start(sn[:], sin3[i * 128:(i + 1) * 128, :])

                sq = pool.tile([128, D], f32, tag="sq")
                ssq = pool.tile([128, 1], f32, tag="ssq")
                nc.scalar.activation(sq[:], x[:], AF.Square, accum_out=ssq[:])
                var = pool.tile([128, 1], f32, tag="var")
                nc.vector.tensor_scalar(var[:], ssq[:], 1.0 / D, EPS,
                                        ALU.mult, ALU.add)
                sd = pool.tile([128, 1], f32, tag="sd")
                nc.scalar.activation(sd[:], var[:], AF.Sqrt)
                s = pool.tile([128, 1], f32, tag="s")
                nc.vector.reciprocal(s[:], sd[:])
                h_t = pool.tile([128, D], f32r, tag="h")
                nc.vector.tensor_scalar_mul(h_t[:], x[:], s[:])

                hT = pool.tile([128, 8, 128], f32r, tag="hT")
                pq = psum.tile([128, 256], f32, tag="pqkv")
                for half in range(2):
                    pt = psum.tile([128, 512], f32r, tag="ptr")
                    for q4 in range(4):
                        dc = half * 4 + q4
                        nc.tensor.transpose(pt[:, q4 * 128:(q4 + 1) * 128],
                                            h_t[:, dc * 128:(dc + 1) * 128],
                                            id_sb[:])
                    nc.vector.tensor_copy(
                        hT[:, half * 4:(half + 1) * 4, :],
                        pt[:].rearrange("p (c n) -> p c n", c=4))
                for dc in range(8):
                    nc.tensor.matmul(pq[:], hT[:, dc, :], wqkv_sb[:, dc, :],
                                     start=(dc == 0), stop=(dc == 7))

                # rope on q(2 heads)+k (groups 0..2); v is group 3
                out_t = pool.tile([128, 256], f32r, tag="out_t")
                g3 = pq[:].rearrange("p (g n) -> p g n", g=4)[:, 0:3, :]
                o3 = out_t[:].rearrange("p (g n) -> p g n", g=4)[:, 0:3, :]
                c1, c2 = g3[:, :, 0:32], g3[:, :, 32:64]
                cs3 = cs[:].rearrange("p (g n) -> p g n", g=3)
                sn3 = sn[:].rearrange("p (g n) -> p g n", g=3)
                tmp = pool.tile([128, 3, 32], f32, tag="tmp")
                nc.vector.tensor_mul(tmp[:], c2, sn3)
                nc.vector.tensor_mul(o3[:, :, 0:32], c1, cs3)
                nc.vector.tensor_sub(o3[:, :, 0:32], o3[:, :, 0:32], tmp[:])
                tmp2 = pool.tile([128, 3, 32], f32, tag="tmp2")
                nc.vector.tensor_mul(tmp2[:], c1, sn3)
                nc.vector.tensor_mul(o3[:, :, 32:64], c2, cs3)
                nc.vector.tensor_add(o3[:, :, 32:64], o3[:, :, 32:64], tmp2[:])

                # v straight from psum
                nc.any.tensor_copy(v_sb[:, i, 0:64], pq[:, 192:256])

                # transposes: q h0, q h1, k  ([128,64] -> [64,128])
                for h in range(2):
                    ptq = psum.tile([64, 128], f32r, tag="ptq")
                    nc.tensor.transpose(ptq[:], out_t[:, h * 64:(h + 1) * 64],
                                        id_sb[:])
                    nc.any.tensor_copy(qT[b][h][:, sblk * 128:(sblk + 1) * 128],
                                       ptq[:])
                ptk = psum.tile([64, 128], f32r, tag="ptk")
                nc.tensor.transpose(ptk[:], out_t[:, 128:192], id_sb[:])
                nc.any.tensor_copy(kT[b][:, sblk * 128:(sblk + 1) * 128], ptk[:])

        # ---------------- phase B: attention per (b, h, s-chunk)
        if 'B' in phases:
         with tc.tile_pool(name="pb", bufs=4) as pool, \
             tc.tile_pool(name="pb_ps", bufs=2, space="PSUM") as psum, \
             tc.tile_pool(name="pb_ps1", bufs=2, space="PSUM") as psum1:
            for b in range(B):
                for h in range(2):
                    for j in range(NSC):
                        ntb = 4 * (j + 1)
                        pctx = psum1.tile([65, 512], f32, tag="pctx")
                        for k in range(ntb):
                            ps = psum.tile([128, 512], f32, tag="ps")
                            nc.tensor.matmul(
                                ps[:], kT[b][:, k * 128:(k + 1) * 128],
                                qT[b][h][:, j * 512:(j + 1) * 512],
                                start=True, stop=True)
                            ex = pool.tile([128, 512], f32r, tag="ex")
                            nc.scalar.activation(ex[:], ps[:], AF.Exp)
                            r = k - 4 * j
                            if r >= 0:  # diagonal region: apply causal mask
                                nc.vector.tensor_mul(ex[:], ex[:],
                                                     mask_sb[:, r, :])
                            nc.tensor.matmul(pctx[:], v_sb[:, b * SBLK + k, :],
                                             ex[:], start=(k == 0),
                                             stop=(k == ntb - 1))
                        rec = pool.tile([65, 512], f32r, tag="rec")
                        with nc.allow_low_precision(reason="f32r rounding of softmax recip"):
                            nc.vector.reciprocal(rec[64:65, :], pctx[64:65, :])
                        pbc = psum.tile([64, 512], f32, tag="pbc")
                        nc.tensor.matmul(pbc[:], ones_sb[64:65, :], rec[64:65, :],
                                         start=True, stop=True)
                        recb = pool.tile([64, 512], f32, tag="recb")
                        nc.scalar.copy(recb[:], pbc[:])
                        nc.vector.tensor_mul(
                            ctxT[b][h][:, j * 512:(j + 1) * 512],
                            pctx[0:64, :], recb[:])

        # ---------------- phase C: out-projection partials
        if 'C' in phases:
         with tc.tile_pool(name="pc", bufs=3) as pool, \
             tc.tile_pool(name="pc_ps", bufs=2, space="PSUM") as psum:
            for i in range(NBLK):
                b, sblk = divmod(i, SBLK)
                o_sb = pool.tile([128, D], f32, tag="o_sb")
                for dn in range(2):
                    po = psum.tile([128, 512], f32, tag="po")
                    for h in range(2):
                        nc.tensor.matmul(
                            po[:],
                            ctxT[b][h][:, sblk * 128:(sblk + 1) * 128],
                            wo_sb[h][:, dn * 512:(dn + 1) * 512],
                            start=(h == 0), stop=(h == 1))
                    nc.any.tensor_copy(o_sb[:, dn * 512:(dn + 1) * 512], po[:])
                nc.sync.dma_start(partial[i * 128:(i + 1) * 128, :], o_sb[:])

    nc.finalize()
    return nc


# ---------------------------------------------------------------- launch 2

def build_moe(cap):
    """Expert FFN: y = (silu(x@Wg) * (x@Wu)) @ Wd for `cap` tokens."""
    assert cap % 256 == 0
    ntc = cap // 256
    nc = bacc.Bacc("TRN2", target_bir_lowering=False, debug=False,
                   num_devices=NCORES)
    xT = nc.dram_tensor("xT", [D, cap], f32, kind="ExternalInput")
    wg = nc.dram_tensor("wg", [D, F], f32, kind="ExternalInput")
    wu = nc.dram_tensor("wu", [D, F], f32, kind="ExternalInput")
    wd = nc.dram_tensor("wd", [F, D], f32, kind="ExternalInput")
    y = nc.dram_tensor("y", [cap, D], f32, kind="ExternalOutput")

    NQ = 8            # F octants
    QF = F // NQ      # 512 f per octant
    QFC = QF // 128   # 4 f-chunks per octant

    with tile.TileContext(nc) as tc, ExitStack() as ctx:
        big = ctx.enter_context(tc.tile_pool(name="big", bufs=1))
        wpool = ctx.enter_context(tc.tile_pool(name="wpool", bufs=2))
        gp = ctx.enter_context(tc.tile_pool(name="gp", bufs=3))
        psg = ctx.enter_context(tc.tile_pool(name="psg", bufs=2, space="PSUM"))
        psy = ctx.enter_context(tc.tile_pool(name="psy", bufs=1, space="PSUM"))

        xT_sb = big.tile([128, 8, cap], f32r)
        nc.gpsimd.dma_start(xT_sb[:], xT.rearrange("(c p) n -> p c n", p=128))
        y_sb = big.tile([128, cap // 128, D], f32)

        for q in range(NQ):
            wg_sb = wpool.tile([128, 8, QF], f32r, tag="wg")
            nc.gpsimd.dma_start(
                wg_sb[:],
                wg[:, q * QF:(q + 1) * QF].rearrange("(c p) n -> p c n", p=128))
            wu_sb = wpool.tile([128, 8, QF], f32r, tag="wu")
            nc.gpsimd.dma_start(
                wu_sb[:],
                wu[:, q * QF:(q + 1) * QF].rearrange("(c p) n -> p c n", p=128))
            wd_sb = wpool.tile([128, QFC, D], f32r, tag="wd")
            nc.gpsimd.dma_start(
                wd_sb[:],
                wd[q * QF:(q + 1) * QF, :].rearrange("(c p) n -> p c n", p=128))

            for t in range(ntc):
                pys = [psy.tile([128, 512], f32, tag=f"py{su}{dn}", name=f"py{su}{dn}")
                       for su in range(2) for dn in range(2)]
                for fc in range(QFC):
                    pg = psg.tile([128, 256], f32, tag="pg")
                    pu = psg.tile([128, 256], f32, tag="pu")
                    for dc in range(8):
                        nc.tensor.matmul(
                            pg[:], wg_sb[:, dc, fc * 128:(fc + 1) * 128],
                            xT_sb[:, dc, t * 256:(t + 1) * 256],
                            start=(dc == 0), stop=(dc == 7))
                    for dc in range(8):
                        nc.tensor.matmul(
                            pu[:], wu_sb[:, dc, fc * 128:(fc + 1) * 128],
                            xT_sb[:, dc, t * 256:(t + 1) * 256],
                            start=(dc == 0), stop=(dc == 7))
                    sl = gp.tile([128, 256], f32, tag="sl")
                    nc.scalar.activation(sl[:], pg[:], AF.Silu)
                    gt = gp.tile([128, 256], f32r, tag="gt")
                    nc.vector.tensor_mul(gt[:], sl[:], pu[:])
                    for su in range(2):
                        for dn in range(2):
                            nc.tensor.matmul(
                                pys[su * 2 + dn][:],
                                gt[:, su * 128:(su + 1) * 128],
                                wd_sb[:, fc, dn * 512:(dn + 1) * 512],
                                start=(fc == 0), stop=(fc == QFC - 1))
                for su in range(2):
                    sub = t * 2 + su
                    for dn in range(2):
                        dst = y_sb[:, sub, dn * 512:(dn + 1) * 512]
                        if q == 0:
                            nc.any.tensor_copy(dst, pys[su * 2 + dn][:])
                        else:
                            nc.vector.tensor_add(dst, dst, pys[su * 2 + dn][:])

        for sub in range(cap // 128):
            nc.sync.dma_start(y[sub * 128:(sub + 1) * 128, :], y_sb[:, sub, :])

    nc.finalize()
    return nc


# ---------------------------------------------------------------- host glue

def _np_rmsnorm(x, w):
    var = np.mean(x * x, axis=-1, keepdims=True, dtype=np.float32)
    return (x * (1.0 / np.sqrt(var + EPS)) * w).astype(np.float32)


def _softmax(x):
    m = x.max(axis=-1, keepdims=True)
    e = np.exp((x - m).astype(np.float32))
    return e / e.sum(axis=-1, keepdims=True, dtype=np.float32)


_ATTN_NC = None
_MOE_NC = {}


def _get_attn_nc():
    global _ATTN_NC
    if _ATTN_NC is None:
        _ATTN_NC = build_attention()
    return _ATTN_NC


def _get_moe_nc(cap):
    if cap not in _MOE_NC:
        _MOE_NC[cap] = build_moe(cap)
    return _MOE_NC[cap]


def kernel(hidden_states, ln1_w, wq, wk, wv, wo, ln2_w, router_w,
           w_gate, w_up, w_down):
    global _LAST_EXEC_NS
    hidden_states = np.asarray(hidden_states, dtype=np.float32)
    ln1_w = np.asarray(ln1_w, dtype=np.float32)
    wq = np.asarray(wq, dtype=np.float32)
    wk = np.asarray(wk, dtype=np.float32)
    wv = np.asarray(wv, dtype=np.float32)
    wo = np.asarray(wo, dtype=np.float32)
    ln2_w = np.asarray(ln2_w, dtype=np.float32)
    router_w = np.asarray(router_w, dtype=np.float32)
    w_gate = np.asarray(w_gate, dtype=np.float32)
    w_up = np.asarray(w_up, dtype=np.float32)
    w_down = np.asarray(w_down, dtype=np.float32)

    hid = hidden_states.reshape(T, D)

    # ------- launch 1: attention
    pos = np.arange(S, dtype=np.float32)
    inv = 1.0 / (ROPE_THETA ** (np.arange(0, HD, 2, dtype=np.float32) / HD))
    ang = pos[:, None] * inv[None, :]
    cosT = np.tile(np.cos(ang).astype(np.float32), (B, 1))
    sinT = np.tile(np.sin(ang).astype(np.float32), (B, 1))
    qs = HD ** -0.5  # fold score scale into q rope tables
    cos3 = np.concatenate([cosT * qs, cosT * qs, cosT], 1).astype(np.float32)
    sin3 = np.concatenate([sinT * qs, sinT * qs, sinT], 1).astype(np.float32)
    ident = np.eye(128, dtype=np.float32)
    # mask4[i, r, j] = 1 if (r*128 + i) <= j  (t-block r within s-chunk)
    ii = np.arange(128)[:, None, None]
    rr = np.arange(4)[None, :, None]
    jj = np.arange(512)[None, None, :]
    mask4 = ((rr * 128 + ii) <= jj).astype(np.float32)

    wql = ln1_w[:, None] * wq
    wkl = ln1_w[:, None] * wk
    wvl = ln1_w[:, None] * wv

    in_maps = []
    for c in range(NCORES):
        kvh = c // 2
        wqkv_c = np.ascontiguousarray(np.concatenate([
            wql[:, c * 128:(c + 1) * 128],
            wkl[:, kvh * 64:(kvh + 1) * 64],
            wvl[:, kvh * 64:(kvh + 1) * 64]], axis=1))
        wo_c = np.ascontiguousarray(
            wo[c * 128:(c + 1) * 128, :].reshape(2, 64, D))
        in_maps.append(dict(hid=hid, wqkv=wqkv_c, wo=wo_c, cos3=cos3,
                            sin3=sin3, ident=ident, mask4=mask4,
                            onesd=np.ones((128, 64), np.float32)))

    nc1 = _get_attn_nc()
    res1 = run_bass_kernel_spmd(nc1, in_maps, core_ids=list(range(NCORES)),
                                trace=_TRACE)
    _LAST_EXEC_NS["attn"] = res1.exec_time_ns
    attn_out = res1.results[0]["partial"].copy()
    for c in range(1, NCORES):
        attn_out += res1.results[c]["partial"]

    hid2 = hid + attn_out

    # ------- host routing (cheap, exact fp32)
    h2 = _np_rmsnorm(hid2, ln2_w)
    logits = h2 @ router_w.T                      # [T, E]
    probs = _softmax(logits)
    top_i = np.argsort(-probs, axis=-1, kind="stable")[:, :TOPK]
    top_p = np.take_along_axis(probs, top_i, axis=-1)
    top_p = top_p / top_p.sum(axis=-1, keepdims=True)

    sel = np.zeros((T, E), dtype=np.float32)
    np.add.at(sel, (np.arange(T)[:, None], top_i), 1.0)
    frac = sel.mean(axis=0) / TOPK
    prob_mean = probs.mean(axis=0)
    balance_loss = np.float32(BAL_COEF * E * np.sum(frac * prob_mean))

    # ------- launch 2: expert FFN
    idx = [np.where(sel[:, e] > 0)[0] for e in range(E)]
    maxc = max(len(ix) for ix in idx)
    cap = max(256, ((maxc + 255) // 256) * 256)

    in_maps2 = []
    for e in range(E):
        xg = np.zeros((cap, D), dtype=np.float32)
        xg[:len(idx[e])] = h2[idx[e]]
        in_maps2.append(dict(
            xT=np.ascontiguousarray(xg.T),
            wg=np.ascontiguousarray(w_gate[e]),
            wu=np.ascontiguousarray(w_up[e]),
            wd=np.ascontiguousarray(w_down[e])))

    nc2 = _get_moe_nc(cap)
    res2 = run_bass_kernel_spmd(nc2, in_maps2, core_ids=list(range(NCORES)),
                                trace=_TRACE)
    _LAST_EXEC_NS["moe"] = res2.exec_time_ns

    gates = np.zeros((T, E), dtype=np.float32)
    np.put_along_axis(gates, top_i, top_p.astype(np.float32), axis=-1)
    moe_out = np.zeros((T, D), dtype=np.float32)
    for e in range(E):
        ye = res2.results[e]["y"][:len(idx[e])]
        moe_out[idx[e]] += gates[idx[e], e][:, None] * ye

    out = (hid2 + moe_out).reshape(B, S, D).astype(np.float32)
    return out, balance_loss


# revision 14
# speedup vs baseline: 1.1066x; 1.0050x over previous
"""Trainium2 Bass kernel for an MoE transformer block (8 NeuronCores).

Strategy:
  Launch 1 (attention, head-parallel): core c computes q-heads {2c, 2c+1}
    (which share kv-head c//2): rmsnorm -> QKV projection (fp32r matmuls)
    -> RoPE -> causal attention -> out-projection partial (its heads'
    slice of wo). Host sums the 8 partials and adds the residual.
  Host: rmsnorm2, router softmax, top-2 routing, gates, balance loss,
    per-expert token gather (exact fp32 numpy; cheap O(T*E) work).
  Launch 2 (MoE, expert-parallel): core c holds expert c's weights and
    computes SwiGLU FFN over the tokens routed to expert c (padded to a
    common capacity). Host scatters gate-weighted outputs back.

All matmuls run in float32r (full PE throughput at free-dim >= 256,
~1e-4 relative error).
"""
import numpy as np
from contextlib import ExitStack

import concourse.bass as bass
import concourse.mybir as mybir
import concourse.tile as tile
import concourse.bacc as bacc
from concourse.bass_utils import run_bass_kernel_spmd

f32 = mybir.dt.float32
f32r = mybir.dt.float32r
AF = mybir.ActivationFunctionType
ALU = mybir.AluOpType

B, S, D = 2, 2048, 1024
T = B * S
H, KVH, HD = 16, 4, 64
E, TOPK, F = 8, 2, 4096
BAL_COEF = 0.01
EPS = 1e-6
ROPE_THETA = 10000.0
NCORES = 8

NBLK = T // 128          # 32 token blocks
SBLK = S // 128          # 16 blocks per batch
NSC = S // 512           # 4 s-chunks per batch

_TRACE = False           # set by test harness for profiling runs
_LAST_EXEC_NS = {}


# ---------------------------------------------------------------- launch 1

def build_attention():
    nc = bacc.Bacc("TRN2", target_bir_lowering=False, debug=False,
                   num_devices=NCORES)
    hid = nc.dram_tensor("hid", [T, D], f32, kind="ExternalInput")
    hidT = nc.dram_tensor("hidT", [D, T], f32, kind="ExternalInput")
    wqkv = nc.dram_tensor("wqkv", [D, 256], f32, kind="ExternalInput")
    wo = nc.dram_tensor("wo", [2, 64, D], f32, kind="ExternalInput")
    cos3 = nc.dram_tensor("cos3", [T, 96], f32, kind="ExternalInput")
    sin3 = nc.dram_tensor("sin3", [T, 96], f32, kind="ExternalInput")
    ident = nc.dram_tensor("ident", [128, 128], f32, kind="ExternalInput")
    mask4 = nc.dram_tensor("mask4", [128, 4, 512], f32, kind="ExternalInput")
    onesd = nc.dram_tensor("onesd", [128, 64], f32, kind="ExternalInput")
    partial = nc.dram_tensor("partial", [T, D], f32, kind="ExternalOutput")

    with tile.TileContext(nc) as tc, ExitStack() as ctx:
        const = ctx.enter_context(tc.tile_pool(name="const", bufs=1))
        big = ctx.enter_context(tc.tile_pool(name="big", bufs=1))

        id_sb = const.tile([128, 128], f32r)
        nc.gpsimd.dma_start(id_sb[:], ident[:])
        wqkv_sb = const.tile([128, 8, 256], f32r)
        nc.gpsimd.dma_start(wqkv_sb[:], wqkv.rearrange("(c p) n -> p c n", p=128))
        wo_sb = [const.tile([64, D], f32r, tag=f"wo{h}", name=f"wo_sb{h}") for h in range(2)]
        for h in range(2):
            nc.gpsimd.dma_start(wo_sb[h][:], wo[h])
        mask_sb = const.tile([128, 4, 512], f32)
        nc.sync.dma_start(mask_sb[:], mask4[:])
        ones_sb = const.tile([65, 64], f32r)
        nc.gpsimd.dma_start(ones_sb[:], onesd[0:65, :])

        # persistent activations
        qT = [[big.tile([64, S], f32r, tag=f"qT{b}{h}", name=f"qT{b}{h}") for h in range(2)]
              for b in range(B)]
        kT = [big.tile([64, S], f32r, tag=f"kT{b}", name=f"kT{b}") for b in range(B)]
        v_sb = big.tile([128, NBLK, 65], f32r)
        nc.gpsimd.dma_start(v_sb[:, :, 64:65], onesd[:, 0:NBLK].unsqueeze(2))
        ctxT = [[big.tile([64, S], f32r, tag=f"ctxT{b}{h}", name=f"ctxT{b}{h}") for h in range(2)]
                for b in range(B)]

        with tc.tile_pool(name="pa", bufs=3) as pool, \
             tc.tile_pool(name="pb", bufs=4) as poolb, \
             tc.tile_pool(name="pc", bufs=3) as poolc, \
             tc.tile_pool(name="ps_a", bufs=1, space="PSUM") as psum_a, \
             tc.tile_pool(name="ps_t", bufs=1, space="PSUM") as psum_t, \
             tc.tile_pool(name="ps_s", bufs=2, space="PSUM") as psum_s, \
             tc.tile_pool(name="ps_x", bufs=2, space="PSUM") as psum_x, \
             tc.tile_pool(name="ps_b1", bufs=1, space="PSUM") as psum_b1, \
             tc.tile_pool(name="ps_c", bufs=1, space="PSUM") as psum_c:

            def phase_a(b):
                for sblk in range(SBLK):
                    i = b * SBLK + sblk
                    x = pool.tile([128, D], f32, tag="x", name="x")
                    nc.sync.dma_start(x[:], hid[i * 128:(i + 1) * 128, :])
                    cs = pool.tile([128, 96], f32, tag="cs", name="cs")
                    nc.sync.dma_start(cs[:], cos3[i * 128:(i + 1) * 128, :])
                    sn = pool.tile([128, 96], f32, tag="sn", name="sn")
                    nc.sync.dma_start(sn[:], sin3[i * 128:(i + 1) * 128, :])

                    sq = pool.tile([128, D], f32, tag="sq", name="sq")
                    ssq = pool.tile([128, 1], f32, tag="ssq", name="ssq")
                    nc.scalar.activation(sq[:], x[:], AF.Square, accum_out=ssq[:])
                    var = pool.tile([128, 1], f32, tag="var", name="var")
                    nc.vector.tensor_scalar(var[:], ssq[:], 1.0 / D, EPS,
                                            ALU.mult, ALU.add)
                    sd = pool.tile([128, 1], f32, tag="sd", name="sd")
                    nc.scalar.activation(sd[:], var[:], AF.Sqrt)
                    s = pool.tile([128, 1], f32, tag="s", name="s")
                    nc.vector.reciprocal(s[:], sd[:])
                    css = pool.tile([128, 96], f32, tag="css", name="css")
                    nc.vector.tensor_scalar_mul(css[:], cs[:], s[:])
                    sns = pool.tile([128, 96], f32, tag="sns", name="sns")
                    nc.vector.tensor_scalar_mul(sns[:], sn[:], s[:])

                    xT = pool.tile([128, 8, 128], f32r, tag="xT", name="xT")
                    nc.gpsimd.dma_start(
                        xT[:], hidT.rearrange("(c p) t -> p c t", p=128)
                        [:, :, i * 128:(i + 1) * 128])
                    pq = psum_a.tile([128, 256], f32, tag="pqkv", name="pq")
                    for dc in range(8):
                        nc.tensor.matmul(pq[:], xT[:, dc, :], wqkv_sb[:, dc, :],
                                         start=(dc == 0), stop=(dc == 7))

                    out_t = pool.tile([128, 256], f32r, tag="out_t", name="out_t")
                    g3 = pq[:].rearrange("p (g n) -> p g n", g=4)[:, 0:3, :]
                    o3 = out_t[:].rearrange("p (g n) -> p g n", g=4)[:, 0:3, :]
                    c1, c2 = g3[:, :, 0:32], g3[:, :, 32:64]
                    cs3 = css[:].rearrange("p (g n) -> p g n", g=3)
                    sn3 = sns[:].rearrange("p (g n) -> p g n", g=3)
                    tmp = pool.tile([128, 3, 32], f32, tag="tmp", name="tmp")
                    nc.vector.tensor_mul(tmp[:], c2, sn3)
                    nc.vector.tensor_mul(o3[:, :, 0:32], c1, cs3)
                    nc.vector.tensor_sub(o3[:, :, 0:32], o3[:, :, 0:32], tmp[:])
                    tmp2 = pool.tile([128, 3, 32], f32, tag="tmp2", name="tmp2")
                    nc.vector.tensor_mul(tmp2[:], c1, sn3)
                    nc.vector.tensor_mul(o3[:, :, 32:64], c2, cs3)
                    nc.vector.tensor_add(o3[:, :, 32:64], o3[:, :, 32:64], tmp2[:])

                    nc.vector.tensor_scalar_mul(v_sb[:, i, 0:64],
                                                pq[:, 192:256], s[:])

                    for h in range(2):
                        ptq = psum_t.tile([64, 128], f32r, tag="ptq", name="ptq")
                        nc.tensor.transpose(ptq[:], out_t[:, h * 64:(h + 1) * 64],
                                            id_sb[:])
                        nc.any.tensor_copy(
                            qT[b][h][:, sblk * 128:(sblk + 1) * 128], ptq[:])
                    ptk = psum_t.tile([64, 128], f32r, tag="ptq", name="ptk")
                    nc.tensor.transpose(ptk[:], out_t[:, 128:192], id_sb[:])
                    nc.any.tensor_copy(kT[b][:, sblk * 128:(sblk + 1) * 128],
                                       ptk[:])

            def phase_b(b):
                for h in range(2):
                    for j in range(NSC):
                        ntb = 4 * (j + 1)
                        pctx = psum_x.tile([65, 512], f32, tag="pctx",
                                           name="pctx")
                        for k in range(ntb):
                            ps = psum_s.tile([128, 512], f32, tag="ps",
                                             name="ps")
                            nc.tensor.matmul(
                                ps[:], kT[b][:, k * 128:(k + 1) * 128],
                                qT[b][h][:, j * 512:(j + 1) * 512],
                                start=True, stop=True)
                            ex = poolb.tile([128, 512], f32r, tag="ex",
                                            name="ex")
                            nc.scalar.activation(ex[:], ps[:], AF.Exp)
                            r = k - 4 * j
                            if r >= 0:
                                nc.vector.tensor_mul(ex[:], ex[:],
                                                     mask_sb[:, r, :])
                            nc.tensor.matmul(pctx[:], v_sb[:, b * SBLK + k, :],
                                             ex[:], start=(k == 0),
                                             stop=(k == ntb - 1))
                        rec = poolb.tile([65, 512], f32r, tag="rec", name="rec")
                        with nc.allow_low_precision(reason="f32r softmax recip"):
                            nc.vector.reciprocal(rec[64:65, :], pctx[64:65, :])
                        pbc = psum_b1.tile([64, 512], f32, tag="pbc", name="pbc")
                        nc.tensor.matmul(pbc[:], ones_sb[64:65, :],
                                         rec[64:65, :], start=True, stop=True)
                        recb = poolb.tile([64, 512], f32, tag="recb",
                                          name="recb")
                        nc.scalar.copy(recb[:], pbc[:])
                        nc.vector.tensor_mul(
                            ctxT[b][h][:, j * 512:(j + 1) * 512],
                            pctx[0:64, :], recb[:])

            def phase_c(b):
                for sblk in range(SBLK):
                    i = b * SBLK + sblk
                    o_sb = poolc.tile([128, D], f32, tag="o_sb", name="o_sb")
                    for dn in range(2):
                        po = psum_c.tile([128, 512], f32, tag="po", name="po")
                        for h in range(2):
                            nc.tensor.matmul(
                                po[:],
                                ctxT[b][h][:, sblk * 128:(sblk + 1) * 128],
                                wo_sb[h][:, dn * 512:(dn + 1) * 512],
                                start=(h == 0), stop=(h == 1))
                        nc.any.tensor_copy(o_sb[:, dn * 512:(dn + 1) * 512],
                                           po[:])
                    nc.sync.dma_start(partial[i * 128:(i + 1) * 128, :], o_sb[:])

            phase_a(0)
            phase_b(0)
            phase_a(1)
            phase_c(0)
            phase_b(1)
            phase_c(1)

    nc.finalize()
    return nc


# ---------------------------------------------------------------- launch 2

def build_moe(cap):
    """Expert FFN: y = (silu(x@Wg) * (x@Wu)) @ Wd for `cap` tokens."""
    assert cap % 256 == 0
    ntc = cap // 256
    nc = bacc.Bacc("TRN2", target_bir_lowering=False, debug=False,
                   num_devices=NCORES)
    xT = nc.dram_tensor("xT", [D, cap], f32, kind="ExternalInput")
    wg = nc.dram_tensor("wg", [D, F], f32, kind="ExternalInput")
    wu = nc.dram_tensor("wu", [D, F], f32, kind="ExternalInput")
    wd = nc.dram_tensor("wd", [F, D], f32, kind="ExternalInput")
    y = nc.dram_tensor("y", [cap, D], f32, kind="ExternalOutput")

    NQ = 8            # F octants
    QF = F // NQ      # 512 f per octant
    QFC = QF // 128   # 4 f-chunks per octant

    with tile.TileContext(nc) as tc, ExitStack() as ctx:
        big = ctx.enter_context(tc.tile_pool(name="big", bufs=1))
        wpool = ctx.enter_context(tc.tile_pool(name="wpool", bufs=2))
        gp = ctx.enter_context(tc.tile_pool(name="gp", bufs=3))
        psg = ctx.enter_context(tc.tile_pool(name="psg", bufs=2, space="PSUM"))
        psy = ctx.enter_context(tc.tile_pool(name="psy", bufs=1, space="PSUM"))

        xT_sb = big.tile([128, 8, cap], f32r)
        nc.gpsimd.dma_start(xT_sb[:], xT.rearrange("(c p) n -> p c n", p=128))
        y_sb = big.tile([128, cap // 128, D], f32)

        for q in range(NQ):
            wg_sb = wpool.tile([128, 8, QF], f32r, tag="wg")
            nc.gpsimd.dma_start(
                wg_sb[:],
                wg[:, q * QF:(q + 1) * QF].rearrange("(c p) n -> p c n", p=128))
            wu_sb = wpool.tile([128, 8, QF], f32r, tag="wu")
            nc.gpsimd.dma_start(
                wu_sb[:],
                wu[:, q * QF:(q + 1) * QF].rearrange("(c p) n -> p c n", p=128))
            wd_sb = wpool.tile([128, QFC, D], f32r, tag="wd")
            nc.gpsimd.dma_start(
                wd_sb[:],
                wd[q * QF:(q + 1) * QF, :].rearrange("(c p) n -> p c n", p=128))

            for t in range(ntc):
                pys = [psy.tile([128, 512], f32, tag=f"py{su}{dn}", name=f"py{su}{dn}")
                       for su in range(2) for dn in range(2)]
                for fc in range(QFC):
                    pg = psg.tile([128, 256], f32, tag="pg")
                    pu = psg.tile([128, 256], f32, tag="pu")
                    for dc in range(8):
                        nc.tensor.matmul(
                            pg[:], wg_sb[:, dc, fc * 128:(fc + 1) * 128],
                            xT_sb[:, dc, t * 256:(t + 1) * 256],
                            start=(dc == 0), stop=(dc == 7))
                    for dc in range(8):
                        nc.tensor.matmul(
                            pu[:], wu_sb[:, dc, fc * 128:(fc + 1) * 128],
                            xT_sb[:, dc, t * 256:(t + 1) * 256],
                            start=(dc == 0), stop=(dc == 7))
                    sl = gp.tile([128, 256], f32, tag="sl")
                    nc.scalar.activation(sl[:], pg[:], AF.Silu)
                    gt = gp.tile([128, 256], f32r, tag="gt")
                    nc.vector.tensor_mul(gt[:], sl[:], pu[:])
                    for su in range(2):
                        for dn in range(2):
                            nc.tensor.matmul(
                                pys[su * 2 + dn][:],
                                gt[:, su * 128:(su + 1) * 128],
                                wd_sb[:, fc, dn * 512:(dn + 1) * 512],
                                start=(fc == 0), stop=(fc == QFC - 1))
                for su in range(2):
                    sub = t * 2 + su
                    for dn in range(2):
                        dst = y_sb[:, sub, dn * 512:(dn + 1) * 512]
                        if q == 0:
                            nc.any.tensor_copy(dst, pys[su * 2 + dn][:])
                        else:
                            nc.vector.tensor_add(dst, dst, pys[su * 2 + dn][:])

        for sub in range(cap // 128):
            nc.sync.dma_start(y[sub * 128:(sub + 1) * 128, :], y_sb[:, sub, :])

    nc.finalize()
    return nc


# ---------------------------------------------------------------- host glue

def _np_rmsnorm(x, w):
    var = np.mean(x * x, axis=-1, keepdims=True, dtype=np.float32)
    return (x * (1.0 / np.sqrt(var + EPS)) * w).astype(np.float32)


def _softmax(x):
    m = x.max(axis=-1, keepdims=True)
    e = np.exp((x - m).astype(np.float32))
    return e / e.sum(axis=-1, keepdims=True, dtype=np.float32)


_ATTN_NC = None
_MOE_NC = {}


def _get_attn_nc():
    global _ATTN_NC
    if _ATTN_NC is None:
        _ATTN_NC = build_attention()
    return _ATTN_NC


def _get_moe_nc(cap):
    if cap not in _MOE_NC:
        _MOE_NC[cap] = build_moe(cap)
    return _MOE_NC[cap]


def kernel(hidden_states, ln1_w, wq, wk, wv, wo, ln2_w, router_w,
           w_gate, w_up, w_down):
    global _LAST_EXEC_NS
    hidden_states = np.asarray(hidden_states, dtype=np.float32)
    ln1_w = np.asarray(ln1_w, dtype=np.float32)
    wq = np.asarray(wq, dtype=np.float32)
    wk = np.asarray(wk, dtype=np.float32)
    wv = np.asarray(wv, dtype=np.float32)
    wo = np.asarray(wo, dtype=np.float32)
    ln2_w = np.asarray(ln2_w, dtype=np.float32)
    router_w = np.asarray(router_w, dtype=np.float32)
    w_gate = np.asarray(w_gate, dtype=np.float32)
    w_up = np.asarray(w_up, dtype=np.float32)
    w_down = np.asarray(w_down, dtype=np.float32)

    hid = hidden_states.reshape(T, D)
    hidT_arr = np.ascontiguousarray(hid.T)

    # ------- launch 1: attention
    pos = np.arange(S, dtype=np.float32)
    inv = 1.0 / (ROPE_THETA ** (np.arange(0, HD, 2, dtype=np.float32) / HD))
    ang = pos[:, None] * inv[None, :]
    cosT = np.tile(np.cos(ang).astype(np.float32), (B, 1))
    sinT = np.tile(np.sin(ang).astype(np.float32), (B, 1))
    qs = HD ** -0.5  # fold score scale into q rope tables
    cos3 = np.concatenate([cosT * qs, cosT * qs, cosT], 1).astype(np.float32)
    sin3 = np.concatenate([sinT * qs, sinT * qs, sinT], 1).astype(np.float32)
    ident = np.eye(128, dtype=np.float32)
    # mask4[i, r, j] = 1 if (r*128 + i) <= j  (t-block r within s-chunk)
    ii = np.arange(128)[:, None, None]
    rr = np.arange(4)[None, :, None]
    jj = np.arange(512)[None, None, :]
    mask4 = ((rr * 128 + ii) <= jj).astype(np.float32)

    wql = ln1_w[:, None] * wq
    wkl = ln1_w[:, None] * wk
    wvl = ln1_w[:, None] * wv

    in_maps = []
    for c in range(NCORES):
        kvh = c // 2
        wqkv_c = np.ascontiguousarray(np.concatenate([
            wql[:, c * 128:(c + 1) * 128],
            wkl[:, kvh * 64:(kvh + 1) * 64],
            wvl[:, kvh * 64:(kvh + 1) * 64]], axis=1))
        wo_c = np.ascontiguousarray(
            wo[c * 128:(c + 1) * 128, :].reshape(2, 64, D))
        in_maps.append(dict(hid=hid, hidT=hidT_arr, wqkv=wqkv_c, wo=wo_c,
                            cos3=cos3, sin3=sin3, ident=ident, mask4=mask4,
                            onesd=np.ones((128, 64), np.float32)))

    nc1 = _get_attn_nc()
    res1 = run_bass_kernel_spmd(nc1, in_maps, core_ids=list(range(NCORES)),
                                trace=_TRACE)
    _LAST_EXEC_NS["attn"] = res1.exec_time_ns
    attn_out = res1.results[0]["partial"].copy()
    for c in range(1, NCORES):
        attn_out += res1.results[c]["partial"]

    hid2 = hid + attn_out

    # ------- host routing (cheap, exact fp32)
    h2 = _np_rmsnorm(hid2, ln2_w)
    logits = h2 @ router_w.T                      # [T, E]
    probs = _softmax(logits)
    top_i = np.argsort(-probs, axis=-1, kind="stable")[:, :TOPK]
    top_p = np.take_along_axis(probs, top_i, axis=-1)
    top_p = top_p / top_p.sum(axis=-1, keepdims=True)

    sel = np.zeros((T, E), dtype=np.float32)
    np.add.at(sel, (np.arange(T)[:, None], top_i), 1.0)
    frac = sel.mean(axis=0) / TOPK
    prob_mean = probs.mean(axis=0)
    balance_loss = np.float32(BAL_COEF * E * np.sum(frac * prob_mean))

    # ------- launch 2: expert FFN
    idx = [np.where(sel[:, e] > 0)[0] for e in range(E)]
    maxc = max(len(ix) for ix in idx)
    cap = max(256, ((maxc + 255) // 256) * 256)

    in_maps2 = []
    for e in range(E):
        xg = np.zeros((cap, D), dtype=np.float32)
        xg[:len(idx[e])] = h2[idx[e]]
        in_maps2.append(dict(
            xT=np.ascontiguousarray(xg.T),
            wg=np.ascontiguousarray(w_gate[e]),
            wu=np.ascontiguousarray(w_up[e]),
            wd=np.ascontiguousarray(w_down[e])))

    nc2 = _get_moe_nc(cap)
    res2 = run_bass_kernel_spmd(nc2, in_maps2, core_ids=list(range(NCORES)),
                                trace=_TRACE)
    _LAST_EXEC_NS["moe"] = res2.exec_time_ns

    gates = np.zeros((T, E), dtype=np.float32)
    np.put_along_axis(gates, top_i, top_p.astype(np.float32), axis=-1)
    moe_out = np.zeros((T, D), dtype=np.float32)
    for e in range(E):
        ye = res2.results[e]["y"][:len(idx[e])]
        moe_out[idx[e]] += gates[idx[e], e][:, None] * ye

    out = (hid2 + moe_out).reshape(B, S, D).astype(np.float32)
    return out, balance_loss
